# revision 1
# baseline (speedup 1.0000x reference)
"""BiLSTM on 8 TRN2 cores — zero-communication batch-sharded design.

Sharding: cores 0-3 run the FORWARD direction, cores 4-7 the BACKWARD
direction (on time-reversed input).  Core r owns batch rows
[16*(r%4), 16*(r%4)+16) and runs the full H=1024 recurrence locally —
no cross-core traffic at all.

Phase 1: xg = x @ W_ih^T + bias as a full-width GEMM (M=128 row tiles
of 128 timesteps x 1 batch row), output stored to DRAM in the
"island" layout the recurrence consumes.

Phase 2 (recurrence): per step, g = h @ W_hh^T is computed as 64
matmuls [K=128, M=16, N=512] packed 4-wide into the PE array with
tile_position column tiling: island J sits at array columns/PSUM
partitions [32J, 32J+16).  Island (bank B, J) is SELF-CONTAINED: it
carries dim chunk k' = 4B+J (128 h-dims) with all four gates as
column blocks [i|f|o|g~] x 128.  Every elementwise op is then a
column slice at identical partitions (the bir verifier requires
samePartitionsAll).  c/h live as [128, 256] in the same island
layout (partition 32J+b, col 128B+u <-> dim 128*(4B+J)+u; partitions
32J+16..32J+32 are dead lanes that never leak).  h -> hT (the next
step's stationary lhsT [128, 16k+b]) is 8 PE transposes (row-packed
4-wide via tile_position) + one DVE psum->sbuf bf16 evac.
"""

import sys
import time

import numpy as np
import ml_dtypes

sys.path.insert(0, "/opt/trn_rl_repo")

import concourse.bass as bass
import concourse.mybir as mybir
from concourse import bacc
from concourse.bass import ds, ts
from concourse.bass_utils import run_bass_kernel_spmd

F32 = mybir.dt.float32
BF16 = mybir.dt.bfloat16
AF = mybir.ActivationFunctionType
OP = mybir.AluOpType
BF16_NP = ml_dtypes.bfloat16

B, S_FULL, I_IN, H = 64, 512, 1024, 1024
BL = 16              # batch rows per core
NCORES = 8
KH = 8               # contraction chunks (1024/128)
GB = [0, 1024, 3072, 2048]   # island J -> gate row base: [i, f, o, g~]


def build(S=S_FULL, simfill=False):
    TSZ = min(S, 128)        # timesteps per phase-1 row tile
    SG = max(1, S // TSZ)    # s-groups
    assert S == TSZ * SG

    nc = bacc.Bacc("TRN2", target_bir_lowering=False, debug=False,
                   num_devices=NCORES)

    # ---- DRAM ----
    xt_d = nc.dram_tensor("xt", [BL, SG, KH, 128, TSZ], BF16,
                          kind="ExternalInput")
    wihT_d = nc.dram_tensor("wihT", [128, KH * 4096], BF16,
                            kind="ExternalInput")
    whhT_d = nc.dram_tensor("whhT", [128, KH * 4096], BF16,
                            kind="ExternalInput")
    bias_d = nc.dram_tensor("bias", [1, 4096], BF16, kind="ExternalInput")
    hout_d = nc.dram_tensor("h", [S, 128, 256], BF16, kind="ExternalOutput")
    sel_d = nc.dram_tensor("sel", [128, 64], BF16, kind="ExternalInput")
    mask_d = nc.dram_tensor("mask", [128, 256], mybir.dt.uint8,
                            kind="ExternalInput")
    xg_d = nc.dram_tensor("xg", [S, 2, 128, 512], BF16, kind="Internal")

    # ---- semaphores ----
    sem = {}
    names = ["sw", "init", "mmp", "evp", "mm", "add", "actg", "cdv",
             "tc", "hdv", "ptr", "htev", "sxt0", "sxt1", "sxg0", "sxg1",
             "sxg2", "sh0", "sh1"]
    names += [f"p1o{q2}{J}" for q2 in range(2) for J in range(4)]
    for nm in names:
        sem[nm] = nc.alloc_semaphore(nm)

    # ---- SBUF ----
    sb = nc.alloc_sbuf_tensor
    wihT_sb = sb("wihT_sb", [128, KH * 4096], BF16).ap()
    whhT_sb = sb("whhT_sb", [128, KH * 4096], BF16).ap()
    bias_sb = sb("bias_sb", [1, 4096], BF16).ap()
    ones_sb = sb("ones_sb", [1, 128], BF16).ap()
    xT = [sb(f"xT{m}", [128, KH * TSZ], BF16).ap() for m in range(2)]
    ot = [sb(f"ot{m}", [TSZ, 512], BF16).ap() for m in range(8)]
    xgb = [sb(f"xgb{m}", [128, 1024], BF16).ap() for m in range(3)]
    gadd = [sb(f"gadd{q}", [128, 1024], F32).ap() for q in range(2)]
    acts = [sb(f"acts{q}", [128, 1024], F32).ap() for q in range(2)]
    c_sb = sb("c_sb", [128, 256], F32).ap()
    t1_sb = sb("t1_sb", [128, 256], F32).ap()
    t2_sb = sb("t2_sb", [128, 256], F32).ap()
    tnc = [sb(f"tnc{q}", [128, 256], F32).ap() for q in range(2)]
    htmp = sb("htmp", [128, 256], BF16).ap()
    hbf = [sb(f"hbf{q}", [128, 256], BF16).ap() for q in range(2)]
    sel_sb = sb("sel_sb", [128, 64], BF16).ap()
    mask_sb = sb("mask_sb", [128, 256], mybir.dt.uint8).ap()
    hT = [sb(f"hT{q}", [128, 128], BF16).ap() for q in range(2)]

    # ---- PSUM: 4 banks phase-1 GEMM, 4 banks recurrence ----
    ap_ = nc.alloc_psum_tensor
    ps1 = [ap_(f"ps1{j}", [128, 512], F32).ap() for j in range(4)]
    psr = [[ap_(f"psr{b_}{p}", [128, 512], F32).ap() for p in range(2)]
           for b_ in range(2)]
    # transpose scratch: alias phase-1 banks (free during recurrence)
    ps_t = [ps1[2][:, 0:128], ps1[3][:, 0:128]]

    # ---- prologue ----
    nc.sync.dma_start(wihT_sb, wihT_d.ap()).then_inc(sem["sw"], 16)
    nc.sync.dma_start(whhT_sb, whhT_d.ap()).then_inc(sem["sw"], 16)
    nc.sync.dma_start(bias_sb, bias_d.ap()).then_inc(sem["sw"], 16)
    nc.sync.dma_start(sel_sb, sel_d.ap()).then_inc(sem["sw"], 16)
    nc.sync.dma_start(mask_sb, mask_d.ap()).then_inc(sem["sw"], 16)

    nc.vector.memset(ones_sb, 1.0).then_inc(sem["init"], 1)
    nc.vector.memset(c_sb, 0.0).then_inc(sem["init"], 1)
    for q in range(2):
        nc.vector.memset(hT[q], 0.0).then_inc(sem["init"], 1)
        nc.vector.memset(psr[0][q], 0.0).then_inc(sem["init"], 1)
        nc.vector.memset(psr[1][q], 0.0).then_inc(sem["init"], 1)

    if simfill:
        zf = nc.alloc_semaphore("zf")
        zt = sb("zt", [128, 1024], BF16).ap()
        nc.vector.memset(zt, 0.0).then_inc(sem["init"], 1)
        nc.sync.wait_ge(sem["init"], 9)
        for t in range(S):
            nc.sync.dma_start(
                xg_d.ap()[t].rearrange("B p n -> p B n"),
                zt.rearrange("p (B n) -> p B n", B=2)).then_inc(zf, 16)
        nc.sync.wait_ge(zf, 16 * S)

    nc.tensor.wait_ge(sem["sw"], 16 * 5)
    nc.tensor.wait_ge(sem["init"], 8)

    # ---- phase 1: xg = x @ wihT + bias, island layout out ----
    # m-tile (b, sg) = TSZ timesteps of one batch row; pass B = dim half.
    tidx = 0      # m-tile counter
    pidx = 0      # (tile, pass) counter
    for b in range(BL):
        for sg in range(SG):
            m2 = tidx % 2
            # in-DMA x^T tile [128, KH*TSZ]
            if tidx >= 2:
                nc.sync.wait_ge(sem["mmp"], 8 * (tidx - 1))
            nc.sync.dma_start(
                xT[m2].rearrange("p (k s) -> p k s", k=KH),
                xt_d.ap()[b, sg].rearrange("k p s -> p k s"),
            ).then_inc(sem[f"sxt{m2}"], 16)
            nc.tensor.wait_ge(sem[f"sxt{m2}"], 16 * (tidx // 2 + 1))
            for Bk in range(2):
                for J in range(4):
                    # psum free: evac of previous use done
                    if pidx >= 1:
                        nc.tensor.wait_ge(sem["evp"], 4 * (pidx - 1) + J + 1)
                    off = Bk * 2048 + J * 512
                    for k in range(KH):
                        nc.tensor.matmul(ps1[J][0:TSZ, :], xT[m2][:, ts(k, TSZ)],
                                         wihT_sb[:, ds(k * 4096 + off, 512)],
                                         start=(k == 0), stop=False)
                    mmi = nc.tensor.matmul(ps1[J][0:TSZ, :], ones_sb[:, 0:TSZ],
                                           bias_sb[:, ds(off, 512)],
                                           start=False, stop=True)
                    mmi.then_inc(sem["mmp"], 1)
                # evac + out-DMA per island
                for J in range(4):
                    oslot = 4 * (pidx % 2) + J
                    nc.vector.wait_ge(sem["mmp"], 4 * pidx + J + 1)
                    if pidx >= 2:
                        nc.vector.wait_ge(sem[f"p1o{pidx % 2}{J}"],
                                          16 * (pidx // 2))
                    nc.vector.tensor_copy(ot[oslot], ps1[J][0:TSZ, :]
                                          ).then_inc(sem["evp"], 1)
                    nc.sync.wait_ge(sem["evp"], 4 * pidx + J + 1)
                    nc.sync.dma_start(
                        xg_d.ap()[ds(sg * TSZ, TSZ), Bk, 32 * J + b, :],
                        ot[oslot]).then_inc(sem[f"p1o{pidx % 2}{J}"], 16)
                pidx += 1
            tidx += 1
    NPASS = pidx

    # ---- phase 2: recurrence ----
    # xg prefetch for steps 0..2
    for q2 in range(2):
        for J in range(4):
            nc.sync.wait_ge(sem[f"p1o{q2}{J}"], 16 * (NPASS // 2))
    for u in range(min(3, S)):
        nc.sync.dma_start(xgb[u].rearrange("p (B n) -> p B n", B=2),
                          xg_d.ap()[u].rearrange("B p n -> p B n")
                          ).then_inc(sem[f"sxg{u % 3}"], 16)

    for t in range(S):
        p = t % 2
        q = t % 2
        m3 = t % 3
        # ---------- SP: xg prefetch t+3 ----------
        if t + 3 < S:
            nc.sync.wait_ge(sem["add"], 2 * t + 2)
            nc.sync.dma_start(xgb[t % 3].rearrange("p (B n) -> p B n", B=2),
                              xg_d.ap()[t + 3].rearrange("B p n -> p B n")
                              ).then_inc(sem[f"sxg{t % 3}"], 16)
        # ---------- PE: 64 MMs, 2 banks x (8k x 4 islands) ----------
        nc.tensor.wait_ge(sem["htev"], t)
        if t >= 2:
            nc.tensor.wait_ge(sem["add"], 2 * t - 2)
        for Bk in range(2):
            for k in range(KH):
                for J in range(4):
                    mmi = nc.tensor.matmul(
                        psr[Bk][p][ds(32 * J, 16), :],
                        hT[(t + 1) % 2][:, ds(16 * k, 16)],
                        whhT_sb[:, ds(k * 4096 + Bk * 2048 + J * 512, 512)],
                        start=(k == 0), stop=(k == KH - 1),
                        tile_position=(0, 32 * J),
                        skip_group_check=True)
            mmi.then_inc(sem["mm"], 1)
        # ---------- DVE: gate adds ----------
        for Bk in range(2):
            nc.vector.wait_ge(sem["mm"], 2 * t + Bk + 1)
            nc.vector.wait_ge(sem[f"sxg{m3}"], 16 * (t // 3 + 1))
            if t >= 2:
                nc.vector.wait_ge(sem["actg"], t - 1)
            nc.vector.tensor_tensor(gadd[q][:, ts(Bk, 512)], psr[Bk][p],
                                    xgb[m3][:, ts(Bk, 512)],
                                    op=OP.add).then_inc(sem["add"], 1)
        # ---------- ACT: sigmoid/tanh gates (cols [i|f|o] / [g~]) ----------
        for Bk in range(2):
            nc.scalar.wait_ge(sem["add"], 2 * t + Bk + 1)
            if t >= 2:
                nc.scalar.wait_ge(sem["hdv"], t - 1)
            nc.scalar.activation(acts[q][:, ds(512 * Bk, 384)],
                                 gadd[q][:, ds(512 * Bk, 384)], AF.Sigmoid)
            ai = nc.scalar.activation(acts[q][:, ds(512 * Bk + 384, 128)],
                                      gadd[q][:, ds(512 * Bk + 384, 128)],
                                      AF.Tanh)
        ai.then_inc(sem["actg"], 1)
        # ---------- DVE: c update (column-sliced, same partitions) ----------
        nc.vector.wait_ge(sem["actg"], t + 1)
        for Bk in range(2):
            nc.vector.tensor_tensor(t2_sb[:, ds(128 * Bk, 128)],
                                    acts[q][:, ds(512 * Bk, 128)],
                                    acts[q][:, ds(512 * Bk + 384, 128)],
                                    op=OP.mult)
            nc.vector.tensor_tensor(t1_sb[:, ds(128 * Bk, 128)],
                                    acts[q][:, ds(512 * Bk + 128, 128)],
                                    c_sb[:, ds(128 * Bk, 128)],
                                    op=OP.mult)
        if t >= 1:
            nc.vector.wait_ge(sem["tc"], t)
        nc.vector.tensor_tensor(c_sb, t1_sb, t2_sb,
                                op=OP.add).then_inc(sem["cdv"], 1)
        # ---------- ACT: tanh(c) ----------
        nc.scalar.wait_ge(sem["cdv"], t + 1)
        nc.scalar.activation(tnc[q], c_sb, AF.Tanh).then_inc(sem["tc"], 1)
        # ---------- DVE: h = o * tanh(c), dead lanes zeroed ----------
        nc.vector.wait_ge(sem["tc"], t + 1)
        if t >= 2:
            nc.vector.wait_ge(sem[f"sh{t % 2}"], 16 * (t // 2))
            nc.vector.wait_ge(sem["ptr"], t - 1)
        for Bk in range(2):
            nc.vector.tensor_tensor(htmp[:, ds(128 * Bk, 128)],
                                    acts[q][:, ds(512 * Bk + 256, 128)],
                                    tnc[q][:, ds(128 * Bk, 128)],
                                    op=OP.mult)
        nc.vector.memset(hbf[q], 0.0)
        nc.vector.copy_predicated(hbf[q], mask_sb, htmp
                                  ).then_inc(sem["hdv"], 1)
        # ---------- PE: hT_k = h^T chunks via selection matmuls ----------
        nc.tensor.wait_ge(sem["hdv"], t + 1)
        if t >= 2:
            nc.tensor.wait_ge(sem["htev"], t - 1)
        for kp in range(KH):
            Bt, Jt = kp // 4, kp % 4
            pti = nc.tensor.matmul(
                ps_t[p][:, ds(16 * kp, 16)],
                hbf[q][:, ds(128 * Bt, 128)],
                sel_sb[:, ds(16 * Jt, 16)],
                start=True, stop=True)
        pti.then_inc(sem["ptr"], 1)
        # ---------- DVE: hT evac (psum -> sbuf bf16) ----------
        nc.vector.wait_ge(sem["ptr"], t + 1)
        nc.vector.tensor_copy(hT[p], ps_t[p]).then_inc(sem["htev"], 1)
        # ---------- SP: h out ----------
        nc.sync.wait_ge(sem["hdv"], t + 1)
        nc.sync.dma_start(hout_d.ap()[t], hbf[q]
                          ).then_inc(sem[f"sh{t % 2}"], 16)

    # ---- epilogue: drain ----
    for par in range(2):
        nc.sync.wait_ge(sem[f"sh{par}"], 16 * ((S + 1 - par) // 2))
    for par in range(3):
        nc.sync.wait_ge(sem[f"sxg{par}"], 16 * ((S - par + 2) // 3))

    nc.compile()
    return nc


_CACHE = {}


def _get(S, simfill=False):
    key = (S, simfill)
    if key not in _CACHE:
        _CACHE[key] = build(S, simfill)
    return _CACHE[key]


def _row_idx():
    """W-row index for xg/psum column (B, J, n): island (B, J) = dim chunk
    k' = 4B+J, cols [i|f|o|g~] x 128 of dims [128k', 128k'+128)."""
    idx = np.empty(4096, np.int64)
    for Bk in range(2):
        for J in range(4):
            kp = 4 * Bk + J
            for X in range(4):
                base = 2048 * Bk + 512 * J + 128 * X
                idx[base:base + 128] = GB[X] + 128 * kp + np.arange(128)
    return idx


def _prep_w(W):
    """W [4096, 1024] f32 -> [128, KH*4096] bf16, layout [p, (k, B, J, n)]."""
    w4 = W[_row_idx()].reshape(2, 4, 512, KH, 128)      # [B, J, n, k, p]
    arr = np.ascontiguousarray(w4.transpose(4, 3, 0, 1, 2)
                               ).reshape(128, KH * 4096)
    return arr.astype(BF16_NP)


def _prep_x(x, S):
    """x [16, S, 1024] f32 -> xt [16, SG, KH, 128, TSZ] bf16."""
    TSZ = min(S, 128)
    SG = S // TSZ
    xb = x.astype(BF16_NP)                                # [b, s, i]
    xb = xb.reshape(BL, SG, TSZ, KH, 128)                 # [b, g, s, k, p]
    return np.ascontiguousarray(xb.transpose(0, 1, 3, 4, 2))


def _selmask():
    sel = np.zeros((128, 64), np.float32)
    for J in range(4):
        for b in range(BL):
            sel[32 * J + b, 16 * J + b] = 1.0
    mask = np.zeros((128, 256), np.uint8)
    mask[(np.arange(128) % 32) < BL] = 1
    return sel.astype(BF16_NP), mask


def _host_shard(inputs, S):
    fx = np.asarray(inputs["forward_x"], np.float32)[:, :S]
    bx = np.asarray(inputs["backward_x"], np.float32)[:, :S][:, ::-1]
    ridx = _row_idx()
    sel, mask = _selmask()
    wmaps = {}
    for d, sfx in (("f", "_f"), ("b", "_b")):
        wmaps[d] = {
            "wihT": _prep_w(np.asarray(inputs[f"W_ih{sfx}"], np.float32)),
            "whhT": _prep_w(np.asarray(inputs[f"W_hh{sfx}"], np.float32)),
            "bias": (np.asarray(inputs[f"b_ih{sfx}"], np.float32)
                     + np.asarray(inputs[f"b_hh{sfx}"], np.float32)
                     )[ridx].reshape(1, 4096).astype(BF16_NP),
        }
    maps = []
    for r in range(NCORES):
        d = "f" if r < 4 else "b"
        x = fx if d == "f" else bx
        q = r % 4
        m = dict(wmaps[d])
        m["sel"] = sel
        m["mask"] = mask
        m["xt"] = _prep_x(x[16 * q:16 * q + 16], S)
        maps.append(m)
    return maps


def _detile(h, S):
    """[S, 128, 256] tiled -> [16, S, 1024]: h[s, 32J+b, 128B+u] = out[b, s,
    128*(4B+J)+u]."""
    a = np.asarray(h, np.float32).reshape(S, 4, 32, 2, 128)[:, :, :BL]
    return np.ascontiguousarray(a.transpose(2, 0, 3, 1, 4)).reshape(BL, S, 1024)


def run(inputs, S=S_FULL, trace=False, **_):
    maps = _host_shard(inputs, S)
    nc = _get(S)
    t0 = time.time()
    res = run_bass_kernel_spmd(nc, maps, core_ids=list(range(NCORES)),
                               trace=trace)
    wall = time.time() - t0
    outs = res.results
    fwd = np.concatenate([_detile(outs[r]["h"], S) for r in range(4)], axis=0)
    bwd = np.concatenate([_detile(outs[r]["h"], S)
                          for r in range(4, 8)], axis=0)[:, ::-1]
    return (fwd, bwd), res, wall


def kernel(**inputs):
    (fwd, bwd), _, _ = run(inputs)
    return fwd, bwd



# revision 3
# speedup vs baseline: 1.1196x; 1.1196x over previous
"""BiLSTM on 8 TRN2 cores — zero-communication batch-sharded design, v2.

Sharding: cores 0-3 run the FORWARD direction, cores 4-7 the BACKWARD
direction (on time-reversed input).  Core r owns batch rows
[16*(r%4), 16*(r%4)+16) and runs the full H=1024 recurrence locally.

Phase 1: xg = x @ W_ih^T + bias as a full-width GEMM (M=128 row tiles
of 128 timesteps x 1 batch row), output stored to DRAM live-lanes-only
as [S, 2, 4, 16, 512] (bank, island, batch, gate-cols).

Phase 2 (recurrence), per step t:
  - xg is INJECTED into PSUM by the PE itself: per island J a matmul
    I64[:, 16J:16J+16] @ xgc (start=True) writes xg into the psum rows
    [32J, 32J+16), so no DVE gate-add is needed and ACT reads gates
    straight from PSUM.
  - g += h @ W_hh^T: 64 matmuls [K=128, M=16, N=512] packed 4-wide via
    tile_position column tiling (island J at psum partitions [32J,32J+16)).
    Groups ordered so bank0 finishes early: [b0k0-3 b1k0-3][b0k4-7 b1k4-7].
  - Per-BANK elementwise pipeline: ACT does tanh(g~), sig(i), sig(f),
    sig(o) as separate ops (finer sem grain), DVE does i*g~, f*c, c-add,
    ACT tanh(c), GPSIMD(Pool) does h = o*tanh(c).  Bank0's tail overlaps
    bank1's matmul stream; the next step's k0-3 groups start as soon as
    bank0's hT chunks are transposed (PE) + evacuated (DVE cast).
Island (bank B, J) carries dim chunk k' = 4B+J with gate columns
[i|f|o|g~] x 128; c/h live as [128, 256] in island layout (partition
32J+b, col 128B+u <-> dim 128*(4B+J)+u).
"""

import sys
import time

import numpy as np
import ml_dtypes

sys.path.insert(0, "/opt/trn_rl_repo")

import concourse.bass as bass
import concourse.mybir as mybir
from concourse import bacc
from concourse.bass import ds, ts
from concourse.bass_utils import run_bass_kernel_spmd

F32 = mybir.dt.float32
BF16 = mybir.dt.bfloat16
AF = mybir.ActivationFunctionType
OP = mybir.AluOpType
BF16_NP = ml_dtypes.bfloat16

B, S_FULL, I_IN, H = 64, 512, 1024, 1024
BL = 16              # batch rows per core
NCORES = 8
KH = 8               # contraction chunks (1024/128)
GB = [0, 1024, 3072, 2048]   # island col-block X -> gate row base: [i, f, o, g~]


def build(S=S_FULL, use_pool=True):
    TSZ = min(S, 128)        # timesteps per phase-1 row tile
    SG = max(1, S // TSZ)    # s-groups
    assert S == TSZ * SG

    nc = bacc.Bacc("TRN2", target_bir_lowering=False, debug=False,
                   num_devices=NCORES)

    # ---- DRAM ----
    xt_d = nc.dram_tensor("xt", [BL, SG, KH, 128, TSZ], BF16,
                          kind="ExternalInput")
    wihT_d = nc.dram_tensor("wihT", [128, KH * 4096], BF16,
                            kind="ExternalInput")
    whhT_d = nc.dram_tensor("whhT", [128, KH * 4096], BF16,
                            kind="ExternalInput")
    bias_d = nc.dram_tensor("bias", [1, 4096], BF16, kind="ExternalInput")
    hout_d = nc.dram_tensor("h", [S, 128, 256], BF16, kind="ExternalOutput")
    sel_d = nc.dram_tensor("sel", [128, 64], BF16, kind="ExternalInput")
    sel2_d = nc.dram_tensor("sel2", [64, 64], BF16, kind="ExternalInput")
    # live lanes only: [t, bank, island, batch, gatecol]
    xg_d = nc.dram_tensor("xg", [S, 2, 4, BL, 512], BF16, kind="Internal")

    # ---- semaphores ----
    sem = {}
    names = ["sw", "init", "mmp", "evp", "injd",
             "sxt0", "sxt1", "sxg0", "sxg1", "sxg2", "sh0", "sh1"]
    names += [f"p1o{q2}{J}" for q2 in range(2) for J in range(4)]
    for b_ in range(2):
        names += [f"mm{b_}", f"gi{b_}", f"gf{b_}", f"ard{b_}", f"cdv{b_}",
                  f"tcs{b_}", f"hdv{b_}", f"ptr{b_}", f"hev{b_}"]
    for nm in names:
        sem[nm] = nc.alloc_semaphore(nm)

    # ---- SBUF ----
    sb = nc.alloc_sbuf_tensor
    wihT_sb = sb("wihT_sb", [128, KH * 4096], BF16).ap()
    whhT_sb = sb("whhT_sb", [128, KH * 4096], BF16).ap()
    bias_sb = sb("bias_sb", [1, 4096], BF16).ap()
    ones_sb = sb("ones_sb", [1, 128], BF16).ap()
    xT = [sb(f"xT{m}", [128, KH * TSZ], BF16).ap() for m in range(2)]
    ot = [sb(f"ot{m}", [TSZ, 512], BF16).ap() for m in range(8)]
    xgc = [sb(f"xgc{m}", [64, 1024], BF16).ap() for m in range(3)]
    acts = [sb(f"acts{q}", [128, 1024], F32).ap() for q in range(2)]
    c_sb = sb("c_sb", [128, 256], F32).ap()
    t1_sb = sb("t1_sb", [128, 256], F32).ap()
    t2_sb = sb("t2_sb", [128, 256], F32).ap()
    tnc = [sb(f"tnc{q}", [128, 256], F32).ap() for q in range(2)]
    htmp = [sb(f"htmp{q}", [128, 256], BF16).ap() for q in range(2)]
    sel_sb = sb("sel_sb", [128, 64], BF16).ap()
    sel2_sb = sb("sel2_sb", [64, 64], BF16).ap()
    hT = [sb(f"hT{q}", [128, 128], BF16).ap() for q in range(2)]

    # ---- PSUM: 4 banks phase-1 GEMM, 4 banks recurrence ----
    ap_ = nc.alloc_psum_tensor
    ps1 = [ap_(f"ps1{j}", [128, 512], F32).ap() for j in range(4)]
    psr = [[ap_(f"psr{b_}{p}", [128, 512], F32).ap() for p in range(2)]
           for b_ in range(2)]
    # transpose scratch: alias phase-1 banks (free during recurrence)
    ps_t = [ps1[2][:, 0:128], ps1[3][:, 0:128]]

    # engine handles (hE = engine used for h-mult: Pool/gpsimd or DVE)
    hE = nc.gpsimd if use_pool else nc.vector

    # ---- prologue ----
    nc.sync.dma_start(wihT_sb, wihT_d.ap()).then_inc(sem["sw"], 16)
    nc.sync.dma_start(whhT_sb, whhT_d.ap()).then_inc(sem["sw"], 16)
    nc.sync.dma_start(bias_sb, bias_d.ap()).then_inc(sem["sw"], 16)
    nc.sync.dma_start(sel_sb, sel_d.ap()).then_inc(sem["sw"], 16)
    nc.sync.dma_start(sel2_sb, sel2_d.ap()).then_inc(sem["sw"], 16)

    nc.vector.memset(ones_sb, 1.0).then_inc(sem["init"], 1)
    nc.vector.memset(c_sb, 0.0).then_inc(sem["init"], 1)
    for q in range(2):
        nc.vector.memset(hT[q], 0.0).then_inc(sem["init"], 1)
        nc.vector.memset(psr[0][q], 0.0).then_inc(sem["init"], 1)
        nc.vector.memset(psr[1][q], 0.0).then_inc(sem["init"], 1)
    NINIT = 8

    for eng in ([nc.tensor, nc.scalar, nc.vector, nc.sync] +
                ([nc.gpsimd] if use_pool else [])):
        eng.wait_ge(sem["sw"], 16 * 5)
        eng.wait_ge(sem["init"], NINIT)

    # ---- phase 1: xg = x @ wihT + bias, live-lane island layout out ----
    tidx = 0      # m-tile counter
    pidx = 0      # (tile, pass) counter
    for b in range(BL):
        for sg in range(SG):
            m2 = tidx % 2
            if tidx >= 2:
                nc.sync.wait_ge(sem["mmp"], 8 * (tidx - 1))
            nc.sync.dma_start(
                xT[m2].rearrange("p (k s) -> p k s", k=KH),
                xt_d.ap()[b, sg].rearrange("k p s -> p k s"),
            ).then_inc(sem[f"sxt{m2}"], 16)
            nc.tensor.wait_ge(sem[f"sxt{m2}"], 16 * (tidx // 2 + 1))
            for Bk in range(2):
                for J in range(4):
                    if pidx >= 1:
                        nc.tensor.wait_ge(sem["evp"], 4 * (pidx - 1) + J + 1)
                    off = Bk * 2048 + J * 512
                    for k in range(KH):
                        nc.tensor.matmul(ps1[J][0:TSZ, :], xT[m2][:, ts(k, TSZ)],
                                         wihT_sb[:, ds(k * 4096 + off, 512)],
                                         start=(k == 0), stop=False)
                    mmi = nc.tensor.matmul(ps1[J][0:TSZ, :], ones_sb[:, 0:TSZ],
                                           bias_sb[:, ds(off, 512)],
                                           start=False, stop=True)
                    mmi.then_inc(sem["mmp"], 1)
                for J in range(4):
                    oslot = 4 * (pidx % 2) + J
                    nc.vector.wait_ge(sem["mmp"], 4 * pidx + J + 1)
                    if pidx >= 2:
                        nc.vector.wait_ge(sem[f"p1o{pidx % 2}{J}"],
                                          16 * (pidx // 2))
                    nc.vector.tensor_copy(ot[oslot], ps1[J][0:TSZ, :]
                                          ).then_inc(sem["evp"], 1)
                    nc.sync.wait_ge(sem["evp"], 4 * pidx + J + 1)
                    nc.sync.dma_start(
                        xg_d.ap()[ds(sg * TSZ, TSZ), Bk, J, b, :],
                        ot[oslot]).then_inc(sem[f"p1o{pidx % 2}{J}"], 16)
                pidx += 1
            tidx += 1
    NPASS = pidx

    # ---- phase 2: recurrence ----
    for q2 in range(2):
        for J in range(4):
            nc.sync.wait_ge(sem[f"p1o{q2}{J}"], 16 * (NPASS // 2))
    # xgc prefetch for steps 0..2
    for u in range(min(3, S)):
        nc.sync.dma_start(
            xgc[u % 3].rearrange("p (B n) -> p B n", B=2),
            xg_d.ap()[u].rearrange("B J b n -> (J b) B n"),
        ).then_inc(sem[f"sxg{u % 3}"], 16)

    def inj(s):
        """PE: inject xg of step s into psr[.][s%2] (start of accum group)."""
        pn = s % 2
        m = s % 3
        nc.tensor.wait_ge(sem[f"sxg{m}"], 16 * (s // 3 + 1))
        if s >= 2:
            nc.tensor.wait_ge(sem["ard0"], s - 1)
            nc.tensor.wait_ge(sem["ard1"], s - 1)
        mi = None
        for Bk in range(2):
            for J in range(4):
                mi = nc.tensor.matmul(
                    psr[Bk][pn][ds(32 * J, 16), :],
                    sel2_sb[:, ds(16 * J, 16)],
                    xgc[m][:, ds(512 * Bk, 512)],
                    start=True, stop=False,
                    tile_position=(0, 32 * J),
                    skip_group_check=True)
        mi.then_inc(sem["injd"], 1)

    inj(0)   # pre-loop injection for step 0

    for t in range(S):
        p = t % 2          # psum parity of this step's gates
        pm = (t + 1) % 2   # hT parity holding h_{t-1}
        q = t % 2          # sbuf parity (acts/tnc/htmp)

        # ---------- PE: h-matmuls, 2 banks x 8k x 4 islands ----------
        # A: k0-3 (needs hT chunks 0..3 = bank0 of t-1)
        nc.tensor.wait_ge(sem["hev0"], t)
        for Bk in range(2):
            for k in range(0, 4):
                for J in range(4):
                    nc.tensor.matmul(
                        psr[Bk][p][ds(32 * J, 16), :],
                        hT[pm][:, ds(16 * k, 16)],
                        whhT_sb[:, ds(k * 4096 + Bk * 2048 + J * 512, 512)],
                        start=False, stop=False,
                        tile_position=(0, 32 * J),
                        skip_group_check=True)
        # B: k4-7 (needs hT chunks 4..7 = bank1 of t-1)
        nc.tensor.wait_ge(sem["hev1"], t)
        for Bk in range(2):
            for k in range(4, 8):
                for J in range(4):
                    mmi = nc.tensor.matmul(
                        psr[Bk][p][ds(32 * J, 16), :],
                        hT[pm][:, ds(16 * k, 16)],
                        whhT_sb[:, ds(k * 4096 + Bk * 2048 + J * 512, 512)],
                        start=False, stop=(k == 7),
                        tile_position=(0, 32 * J),
                        skip_group_check=True)
            mmi.then_inc(sem[f"mm{Bk}"], 1)
        # C: inject xg for step t+1 (fills PE idle while tail runs)
        if t + 1 < S:
            inj(t + 1)
        # D/E: transposes of h_t per bank -> ps_t[p]
        for Bk in range(2):
            nc.tensor.wait_ge(sem[f"hdv{Bk}"], t + 1)
            for Jt in range(4):
                kp = 4 * Bk + Jt
                pti = nc.tensor.matmul(
                    ps_t[p][:, ds(16 * kp, 16)],
                    htmp[q][:, ds(128 * Bk, 128)],
                    sel_sb[:, ds(16 * Jt, 16)],
                    start=True, stop=True)
            pti.then_inc(sem[f"ptr{Bk}"], 1)

        # ---------- ACT: per-bank gate activations + tanh(c) ----------
        for Bk in range(2):
            po = psr[Bk][p]
            ao = 512 * Bk
            nc.scalar.wait_ge(sem[f"mm{Bk}"], t + 1)
            if t >= 2:
                nc.scalar.wait_ge(sem[f"hdv{Bk}"], t - 1)
            nc.scalar.activation(acts[q][:, ds(ao + 384, 128)],
                                 po[:, ds(384, 128)], AF.Tanh)
            nc.scalar.activation(acts[q][:, ds(ao, 128)],
                                 po[:, ds(0, 128)], AF.Sigmoid
                                 ).then_inc(sem[f"gi{Bk}"], 1)
            nc.scalar.activation(acts[q][:, ds(ao + 128, 128)],
                                 po[:, ds(128, 128)], AF.Sigmoid
                                 ).then_inc(sem[f"gf{Bk}"], 1)
            nc.scalar.activation(acts[q][:, ds(ao + 256, 128)],
                                 po[:, ds(256, 128)], AF.Sigmoid
                                 ).then_inc(sem[f"ard{Bk}"], 1)
            # tanh(c) for this bank once DVE finished the c update
            nc.scalar.wait_ge(sem[f"cdv{Bk}"], t + 1)
            nc.scalar.activation(tnc[q][:, ds(128 * Bk, 128)],
                                 c_sb[:, ds(128 * Bk, 128)], AF.Tanh
                                 ).then_inc(sem[f"tcs{Bk}"], 1)

        # ---------- DVE: c update per bank + hT evacs ----------
        for Bk in range(2):
            co = 128 * Bk
            ao = 512 * Bk
            nc.vector.wait_ge(sem[f"gi{Bk}"], t + 1)
            nc.vector.tensor_tensor(t2_sb[:, ds(co, 128)],
                                    acts[q][:, ds(ao, 128)],
                                    acts[q][:, ds(ao + 384, 128)],
                                    op=OP.mult)
            nc.vector.wait_ge(sem[f"gf{Bk}"], t + 1)
            if t >= 1:
                nc.vector.wait_ge(sem[f"tcs{Bk}"], t)
            nc.vector.tensor_tensor(t1_sb[:, ds(co, 128)],
                                    acts[q][:, ds(ao + 128, 128)],
                                    c_sb[:, ds(co, 128)],
                                    op=OP.mult)
            nc.vector.tensor_tensor(c_sb[:, ds(co, 128)],
                                    t1_sb[:, ds(co, 128)],
                                    t2_sb[:, ds(co, 128)],
                                    op=OP.add).then_inc(sem[f"cdv{Bk}"], 1)
        # ---------- Pool/GPSIMD: h = o * tanh(c) per bank ----------
        # (emitted before the DVE evacs so the use_pool=False fallback,
        # which puts these on DVE too, keeps a deadlock-free order)
        for Bk in range(2):
            co = 128 * Bk
            ao = 512 * Bk
            hE.wait_ge(sem[f"tcs{Bk}"], t + 1)
            if t >= 2:
                hE.wait_ge(sem[f"ptr{Bk}"], t - 1)
                if Bk == 0:
                    hE.wait_ge(sem[f"sh{q}"], 16 * (t // 2))
            hE.tensor_tensor(htmp[q][:, ds(co, 128)],
                             acts[q][:, ds(ao + 256, 128)],
                             tnc[q][:, ds(co, 128)],
                             op=OP.mult).then_inc(sem[f"hdv{Bk}"], 1)

        # ---------- DVE: hT evacs (psum -> sbuf bf16) ----------
        for Bk in range(2):
            nc.vector.wait_ge(sem[f"ptr{Bk}"], t + 1)
            nc.vector.tensor_copy(hT[p][:, ds(64 * Bk, 64)],
                                  ps_t[p][:, ds(64 * Bk, 64)]
                                  ).then_inc(sem[f"hev{Bk}"], 1)

        # ---------- SP: xgc prefetch t+3, h out ----------
        if t + 3 < S:
            nc.sync.wait_ge(sem["injd"], t + 1)
            nc.sync.dma_start(
                xgc[(t + 3) % 3].rearrange("p (B n) -> p B n", B=2),
                xg_d.ap()[t + 3].rearrange("B J b n -> (J b) B n"),
            ).then_inc(sem[f"sxg{(t + 3) % 3}"], 16)
        nc.sync.wait_ge(sem["hdv0"], t + 1)
        nc.sync.wait_ge(sem["hdv1"], t + 1)
        nc.sync.dma_start(hout_d.ap()[t], htmp[q]
                          ).then_inc(sem[f"sh{q}"], 16)

    # ---- epilogue: drain ----
    for par in range(2):
        nc.sync.wait_ge(sem[f"sh{par}"], 16 * ((S + 1 - par) // 2))
    for par in range(3):
        nc.sync.wait_ge(sem[f"sxg{par}"], 16 * ((S - par + 2) // 3))

    nc.compile()
    return nc


_CACHE = {}


def _get(S, use_pool=True):
    key = (S, use_pool)
    if key not in _CACHE:
        _CACHE[key] = build(S, use_pool)
    return _CACHE[key]


def _row_idx():
    """W-row index for xg/psum column (B, J, n): island (B, J) = dim chunk
    k' = 4B+J, cols [i|f|o|g~] x 128 of dims [128k', 128k'+128)."""
    idx = np.empty(4096, np.int64)
    for Bk in range(2):
        for J in range(4):
            kp = 4 * Bk + J
            for X in range(4):
                base = 2048 * Bk + 512 * J + 128 * X
                idx[base:base + 128] = GB[X] + 128 * kp + np.arange(128)
    return idx


def _prep_w(W):
    """W [4096, 1024] f32 -> [128, KH*4096] bf16, layout [p, (k, B, J, n)]."""
    w4 = W[_row_idx()].reshape(2, 4, 512, KH, 128)      # [B, J, n, k, p]
    arr = np.ascontiguousarray(w4.transpose(4, 3, 0, 1, 2)
                               ).reshape(128, KH * 4096)
    return arr.astype(BF16_NP)


def _prep_x(x, S):
    """x [16, S, 1024] f32 -> xt [16, SG, KH, 128, TSZ] bf16."""
    TSZ = min(S, 128)
    SG = S // TSZ
    xb = x.astype(BF16_NP)                                # [b, s, i]
    xb = xb.reshape(BL, SG, TSZ, KH, 128)                 # [b, g, s, k, p]
    return np.ascontiguousarray(xb.transpose(0, 1, 3, 4, 2))


def _selmats():
    sel = np.zeros((128, 64), np.float32)
    for J in range(4):
        for b in range(BL):
            sel[32 * J + b, 16 * J + b] = 1.0
    sel2 = np.eye(64, dtype=np.float32)
    return sel.astype(BF16_NP), sel2.astype(BF16_NP)


def _host_shard(inputs, S):
    fx = np.asarray(inputs["forward_x"], np.float32)[:, :S]
    bx = np.asarray(inputs["backward_x"], np.float32)[:, :S][:, ::-1]
    ridx = _row_idx()
    sel, sel2 = _selmats()
    wmaps = {}
    for d, sfx in (("f", "_f"), ("b", "_b")):
        wmaps[d] = {
            "wihT": _prep_w(np.asarray(inputs[f"W_ih{sfx}"], np.float32)),
            "whhT": _prep_w(np.asarray(inputs[f"W_hh{sfx}"], np.float32)),
            "bias": (np.asarray(inputs[f"b_ih{sfx}"], np.float32)
                     + np.asarray(inputs[f"b_hh{sfx}"], np.float32)
                     )[ridx].reshape(1, 4096).astype(BF16_NP),
        }
    maps = []
    for r in range(NCORES):
        d = "f" if r < 4 else "b"
        x = fx if d == "f" else bx
        qb = r % 4
        m = dict(wmaps[d])
        m["sel"] = sel
        m["sel2"] = sel2
        m["xt"] = _prep_x(x[16 * qb:16 * qb + 16], S)
        maps.append(m)
    return maps


def _detile(h, S):
    """[S, 128, 256] tiled -> [16, S, 1024]: h[s, 32J+b, 128B+u] = out[b, s,
    128*(4B+J)+u]."""
    a = np.asarray(h, np.float32).reshape(S, 4, 32, 2, 128)[:, :, :BL]
    return np.ascontiguousarray(a.transpose(2, 0, 3, 1, 4)).reshape(BL, S, 1024)


def run(inputs, S=S_FULL, trace=False, use_pool=True, **_):
    maps = _host_shard(inputs, S)
    nc = _get(S, use_pool)
    t0 = time.time()
    res = run_bass_kernel_spmd(nc, maps, core_ids=list(range(NCORES)),
                               trace=trace)
    wall = time.time() - t0
    outs = res.results
    fwd = np.concatenate([_detile(outs[r]["h"], S) for r in range(4)], axis=0)
    bwd = np.concatenate([_detile(outs[r]["h"], S)
                          for r in range(4, 8)], axis=0)[:, ::-1]
    return (fwd, bwd), res, wall


def kernel(**inputs):
    (fwd, bwd), _, _ = run(inputs)
    return fwd, bwd


# revision 8
# speedup vs baseline: 1.5906x; 1.4207x over previous
"""BiLSTM on 8 TRN2 cores — zero-communication batch-sharded design, v3.

Sharding: cores 0-3 run the FORWARD direction, cores 4-7 the BACKWARD
direction (on time-reversed input).  Core r owns batch rows
[16*(r%4), 16*(r%4)+16) and runs the full H=1024 recurrence locally.

Phase 1 (xg = x @ W_ih^T + bias) is decomposed into 512 "pieces"
(tile b x seq-group sg x bank Bk x island J: 8 matmuls [K=128, M=128,
N=512], bias folded into the DVE evacuation via a host-built broadcast
buffer).  Only seq-group 0 (+4 lookahead pieces) runs up front; the
remaining pieces are interleaved ONE PER STEP into the recurrence's
PE idle window, so phase 1 almost vanishes from the serial span.

Phase 2 (recurrence), per step t the PE program is
  [A: h-mm k0-3 both banks] [E': transpose bank1 of h_{t-1}]
  [inj xg_{t+1} into psum]  [B: h-mm k4-7 both banks]
  [phase-1 piece]           [D: transpose bank0 of h_t]
so next-step matmuls overlap the current step's elementwise tail.
xg is injected into PSUM by the PE itself (identity matmul,
start=True); ACT reads gates straight from PSUM (tanh g~, sig i, sig
f, sig o as separate ops for fine-grained semaphores), DVE does the
c/h updates per bank, ACT also evacuates the transposed hT (psum ->
sbuf bf16 copy).  Island (bank B, J) = dim chunk k' = 4B+J, gate
columns [i|f|o|g~] x 128; c/h live as [128, 256] in island layout.
"""

import sys
import time

import numpy as np
import ml_dtypes

sys.path.insert(0, "/opt/trn_rl_repo")

import concourse.bass as bass
import concourse.mybir as mybir
from concourse import bacc
from concourse.bass import ds, ts
from concourse.bass_utils import run_bass_kernel_spmd

F32 = mybir.dt.float32
BF16 = mybir.dt.bfloat16
AF = mybir.ActivationFunctionType
OP = mybir.AluOpType
BF16_NP = ml_dtypes.bfloat16

B, S_FULL, I_IN, H = 64, 512, 1024, 1024
BL = 16              # batch rows per core
NCORES = 8
KH = 8               # contraction chunks (1024/128)
GB = [0, 1024, 3072, 2048]   # island col-block X -> gate row base: [i, f, o, g~]


def build(S=S_FULL, use_pool=False):
    TSZ = min(S, 128)        # timesteps per phase-1 row tile
    SG = max(1, S // TSZ)    # s-groups
    assert S == TSZ * SG

    nc = bacc.Bacc("TRN2", target_bir_lowering=False, debug=False,
                   num_devices=NCORES)

    # ---- DRAM ----
    xt_d = nc.dram_tensor("xt", [BL, SG, KH, 128, TSZ], BF16,
                          kind="ExternalInput")
    wihT_d = nc.dram_tensor("wihT", [128, KH * 4096], BF16,
                            kind="ExternalInput")
    whhT_d = nc.dram_tensor("whhT", [128, KH * 4096], BF16,
                            kind="ExternalInput")
    biasb_d = nc.dram_tensor("biasb", [128, 4096], BF16, kind="ExternalInput")
    hout_d = nc.dram_tensor("h", [S, 128, 256], BF16, kind="ExternalOutput")
    sel_d = nc.dram_tensor("sel", [128, 64], BF16, kind="ExternalInput")
    sel2_d = nc.dram_tensor("sel2", [64, 64], BF16, kind="ExternalInput")
    # live lanes only: [t, bank, island, batch, gatecol]
    xg_d = nc.dram_tensor("xg", [S, 2, 4, BL, 512], BF16, kind="Internal")

    # ---- semaphores ----
    sem = {}
    names = ["sw", "init", "mmp", "evp", "p1s", "injd",
             "sxt0", "sxt1", "sxg0", "sxg1", "sxg2", "sh0", "sh1"]
    for b_ in range(2):
        names += [f"mm{b_}", f"gi{b_}", f"gf{b_}", f"ard{b_}", f"cdv{b_}",
                  f"tcs{b_}", f"hdv{b_}", f"ptr{b_}", f"hev{b_}"]
    for nm in names:
        sem[nm] = nc.alloc_semaphore(nm)

    # ---- SBUF ----
    sb = nc.alloc_sbuf_tensor
    wihT_sb = sb("wihT_sb", [128, KH * 4096], BF16).ap()
    whhT_sb = sb("whhT_sb", [128, KH * 4096], BF16).ap()
    biasb_sb = sb("biasb_sb", [128, 4096], BF16).ap()
    xT = [sb(f"xT{m}", [128, KH * TSZ], BF16).ap() for m in range(2)]
    ot = [sb(f"ot{m}", [TSZ, 512], BF16).ap() for m in range(4)]
    xgc = [sb(f"xgc{m}", [64, 1024], BF16).ap() for m in range(3)]
    acts = [sb(f"acts{q}", [128, 1024], F32).ap() for q in range(2)]
    c_sb = sb("c_sb", [128, 256], F32).ap()
    t1_sb = sb("t1_sb", [128, 256], F32).ap()
    t2_sb = sb("t2_sb", [128, 256], F32).ap()
    tnc = [sb(f"tnc{q}", [128, 256], F32).ap() for q in range(2)]
    htmp = [sb(f"htmp{q}", [128, 256], BF16).ap() for q in range(2)]
    sel_sb = sb("sel_sb", [128, 64], BF16).ap()
    sel2_sb = sb("sel2_sb", [64, 64], BF16).ap()
    hT = [sb(f"hT{q}", [128, 128], BF16).ap() for q in range(2)]

    # ---- PSUM: 2 banks phase-1 pieces, 2 transpose scratch, 4 recurrence ----
    ap_ = nc.alloc_psum_tensor
    ps1 = [ap_(f"ps1{j}", [128, 512], F32).ap() for j in range(2)]
    ps_t = [ap_(f"pst{j}", [128, 128], F32).ap() for j in range(2)]
    psr = [[ap_(f"psr{b_}{p}", [128, 512], F32).ap() for p in range(2)]
           for b_ in range(2)]

    hE = nc.gpsimd if use_pool else nc.vector

    # ---- prologue ----
    nc.sync.dma_start(wihT_sb, wihT_d.ap()).then_inc(sem["sw"], 16)
    nc.sync.dma_start(whhT_sb, whhT_d.ap()).then_inc(sem["sw"], 16)
    nc.sync.dma_start(biasb_sb, biasb_d.ap()).then_inc(sem["sw"], 16)
    nc.sync.dma_start(sel_sb, sel_d.ap()).then_inc(sem["sw"], 16)
    nc.sync.dma_start(sel2_sb, sel2_d.ap()).then_inc(sem["sw"], 16)

    nc.vector.memset(c_sb, 0.0).then_inc(sem["init"], 1)
    for q in range(2):
        nc.vector.memset(hT[q], 0.0).then_inc(sem["init"], 1)
        nc.vector.memset(psr[0][q], 0.0).then_inc(sem["init"], 1)
        nc.vector.memset(psr[1][q], 0.0).then_inc(sem["init"], 1)
    NINIT = 7

    for eng in ([nc.tensor, nc.scalar, nc.vector, nc.sync] +
                ([nc.gpsimd] if use_pool else [])):
        eng.wait_ge(sem["sw"], 16 * 5)
        eng.wait_ge(sem["init"], NINIT)

    # ---- phase 1 pieces ----
    # piece list, sg-major so seq-group g completes before steps 128g need it
    plist = [(b, sg, Bk, J) for sg in range(SG) for b in range(BL)
             for Bk in range(2) for J in range(4)]
    NP_TOT = len(plist)
    # xT tile loads: tile index i = sg*BL + b, consumed piece-major (8/tile)
    tload = [0]   # next tile to load
    pcnt = [0]    # pieces emitted

    def load_tile(i):
        """SP: DMA xT tile i (if any left)."""
        if i >= SG * BL:
            return
        sg, b = divmod(i, BL)
        m2 = i % 2
        if i >= 2:
            nc.sync.wait_ge(sem["mmp"], 8 * (i - 1))  # piece mms of tile i-2
        nc.sync.dma_start(
            xT[m2].rearrange("p (k s) -> p k s", k=KH),
            xt_d.ap()[b, sg].rearrange("k p s -> p k s"),
        ).then_inc(sem[f"sxt{m2}"], 16)

    def piece_mm(i):
        """PE: 8 matmuls of piece i into ps1[i%2]."""
        b, sg, Bk, J = plist[i]
        tile = sg * BL + b
        m2 = tile % 2
        if i % 8 == 0:
            nc.tensor.wait_ge(sem[f"sxt{m2}"], 16 * (tile // 2 + 1))
        if i >= 2:
            nc.tensor.wait_ge(sem["evp"], i - 1)   # evac of piece i-2 done
        off = Bk * 2048 + J * 512
        for k in range(KH):
            mmi = nc.tensor.matmul(ps1[i % 2][0:TSZ, :],
                                   xT[m2][:, ts(k, TSZ)],
                                   wihT_sb[:, ds(k * 4096 + off, 512)],
                                   start=(k == 0), stop=(k == KH - 1))
        mmi.then_inc(sem["mmp"], 1)

    def piece_evac(i):
        """DVE: ps1 + bias -> ot (bf16)."""
        b, sg, Bk, J = plist[i]
        off = Bk * 2048 + J * 512
        nc.vector.wait_ge(sem["mmp"], i + 1)
        if i >= 4:
            nc.vector.wait_ge(sem["p1s"], 16 * (i - 3))  # ot slot free
        nc.vector.tensor_tensor(ot[i % 4], ps1[i % 2][0:TSZ, :],
                                biasb_sb[0:TSZ, ds(off, 512)],
                                op=OP.add).then_inc(sem["evp"], 1)

    def piece_dma(i):
        """SP: ot -> xg_d."""
        b, sg, Bk, J = plist[i]
        nc.sync.wait_ge(sem["evp"], i + 1)
        nc.sync.dma_start(
            xg_d.ap()[ds(sg * TSZ, TSZ), Bk, J, b, :],
            ot[i % 4]).then_inc(sem["p1s"], 16)

    def emit_piece():
        i = pcnt[0]
        if i >= NP_TOT:
            return
        if i % 8 == 0:
            load_tile(tload[0])   # prefetch: tile loads run 2 tiles ahead
            tload[0] += 1
        piece_mm(i)
        piece_evac(i)
        piece_dma(i)
        pcnt[0] += 1

    def emit_piece_pe():
        """PE part only (loop body); DVE/SP parts follow later in the
        iteration so they don't block the recurrence tail."""
        i = pcnt[0]
        if i >= NP_TOT:
            return None
        piece_mm(i)
        pcnt[0] += 1
        return i

    def emit_piece_rest(i):
        if i is None:
            return
        piece_evac(i)
        if i % 8 == 0:
            load_tile(tload[0])   # prefetch next tile (used 8 pieces later)
            tload[0] += 1
        piece_dma(i)

    # prologue pieces: seq-group 0 fully + 4 lookahead (if more segs exist)
    load_tile(0); tload[0] = 1
    NPRO = min(NP_TOT, BL * 8 + 4) if SG > 1 else NP_TOT
    for _ in range(NPRO):
        emit_piece()

    # ---- phase 2: recurrence ----
    def xgc_load(s):
        """SP: load xg of step s into xgc[s%3]."""
        nc.sync.wait_ge(sem["p1s"], 16 * BL * 8 * (s // TSZ + 1))
        if s >= 3:
            nc.sync.wait_ge(sem["injd"], s - 2)   # inj(s-3) read the slot
        nc.sync.dma_start(
            xgc[s % 3].rearrange("p (B n) -> p B n", B=2),
            xg_d.ap()[s].rearrange("B J b n -> (J b) B n"),
        ).then_inc(sem[f"sxg{s % 3}"], 16)

    for u in range(min(3, S)):
        xgc_load(u)

    def inj(s):
        """PE: inject xg of step s into psr[.][s%2] (start of accum group)."""
        pn = s % 2
        m = s % 3
        nc.tensor.wait_ge(sem[f"sxg{m}"], 16 * (s // 3 + 1))
        if s >= 2:
            nc.tensor.wait_ge(sem["ard0"], s - 1)
            nc.tensor.wait_ge(sem["ard1"], s - 1)
        mi = None
        for Bk in range(2):
            for J in range(4):
                mi = nc.tensor.matmul(
                    psr[Bk][pn][ds(32 * J, 16), :],
                    sel2_sb[:, ds(16 * J, 16)],
                    xgc[m][:, ds(512 * Bk, 512)],
                    start=True, stop=False,
                    tile_position=(0, 32 * J),
                    skip_group_check=True)
        mi.then_inc(sem["injd"], 1)

    inj(0)   # pre-loop injection for step 0

    def transp(t, Bk):
        """PE: transpose bank Bk of h_t into ps_t[t%2] cols [64Bk, 64Bk+64)."""
        q = t % 2
        nc.tensor.wait_ge(sem[f"hdv{Bk}"], t + 1)
        for Jt in range(4):
            kp = 4 * Bk + Jt
            pti = nc.tensor.matmul(
                ps_t[q][:, ds(16 * kp, 16)],
                htmp[q][:, ds(128 * Bk, 128)],
                sel_sb[:, ds(16 * Jt, 16)],
                start=True, stop=True)
        pti.then_inc(sem[f"ptr{Bk}"], 1)

    for t in range(S):
        p = t % 2          # psum parity of this step's gates
        pm = (t + 1) % 2   # hT parity holding h_{t-1}
        q = t % 2          # sbuf parity (acts/tnc/htmp)

        # ================= PE =================
        # A: k0-3 (needs hT chunks 0..3 = evac of bank0 of t-1)
        nc.tensor.wait_ge(sem["hev0"], t)
        for Bk in range(2):
            for k in range(0, 4):
                for J in range(4):
                    nc.tensor.matmul(
                        psr[Bk][p][ds(32 * J, 16), :],
                        hT[pm][:, ds(16 * k, 16)],
                        whhT_sb[:, ds(k * 4096 + Bk * 2048 + J * 512, 512)],
                        start=False, stop=False,
                        tile_position=(0, 32 * J),
                        skip_group_check=True)
        # E': transpose bank1 of h_{t-1}
        if t >= 1:
            transp(t - 1, 1)
        # inj xg for step t+1
        if t + 1 < S:
            inj(t + 1)
        # B: k4-7 (needs hT chunks 4..7 = evac of bank1 of t-1)
        nc.tensor.wait_ge(sem["hev1"], t)
        for Bk in range(2):
            for k in range(4, 8):
                for J in range(4):
                    mmi = nc.tensor.matmul(
                        psr[Bk][p][ds(32 * J, 16), :],
                        hT[pm][:, ds(16 * k, 16)],
                        whhT_sb[:, ds(k * 4096 + Bk * 2048 + J * 512, 512)],
                        start=False, stop=(k == 7),
                        tile_position=(0, 32 * J),
                        skip_group_check=True)
            mmi.then_inc(sem[f"mm{Bk}"], 1)
        # phase-1 piece fills the wait for h_t
        pi = emit_piece_pe()
        # D: transpose bank0 of h_t
        if t + 1 < S:
            transp(t, 0)

        # ================= ACT =================
        # evac bank1 of h_{t-1}: ps_t -> hT (bf16)
        if t >= 1:
            nc.scalar.wait_ge(sem["ptr1"], t)
            nc.scalar.copy(hT[pm][:, ds(64, 64)], ps_t[pm][:, ds(64, 64)]
                           ).then_inc(sem["hev1"], 1)
        for Bk in range(2):
            po = psr[Bk][p]
            ao = 512 * Bk
            nc.scalar.wait_ge(sem[f"mm{Bk}"], t + 1)
            if t >= 2:
                nc.scalar.wait_ge(sem[f"hdv{Bk}"], t - 1)
            nc.scalar.activation(acts[q][:, ds(ao + 384, 128)],
                                 po[:, ds(384, 128)], AF.Tanh)
            nc.scalar.activation(acts[q][:, ds(ao, 128)],
                                 po[:, ds(0, 128)], AF.Sigmoid
                                 ).then_inc(sem[f"gi{Bk}"], 1)
            nc.scalar.activation(acts[q][:, ds(ao + 128, 128)],
                                 po[:, ds(128, 128)], AF.Sigmoid
                                 ).then_inc(sem[f"gf{Bk}"], 1)
            nc.scalar.activation(acts[q][:, ds(ao + 256, 128)],
                                 po[:, ds(256, 128)], AF.Sigmoid
                                 ).then_inc(sem[f"ard{Bk}"], 1)
            nc.scalar.wait_ge(sem[f"cdv{Bk}"], t + 1)
            nc.scalar.activation(tnc[q][:, ds(128 * Bk, 128)],
                                 c_sb[:, ds(128 * Bk, 128)], AF.Tanh
                                 ).then_inc(sem[f"tcs{Bk}"], 1)
        # evac bank0 of h_t
        if t + 1 < S:
            nc.scalar.wait_ge(sem["ptr0"], t + 1)
            nc.scalar.copy(hT[p][:, ds(0, 64)], ps_t[p][:, ds(0, 64)]
                           ).then_inc(sem["hev0"], 1)

        # ================= DVE =================
        for Bk in range(2):
            co = 128 * Bk
            ao = 512 * Bk
            nc.vector.wait_ge(sem[f"gi{Bk}"], t + 1)
            nc.vector.tensor_tensor(t2_sb[:, ds(co, 128)],
                                    acts[q][:, ds(ao, 128)],
                                    acts[q][:, ds(ao + 384, 128)],
                                    op=OP.mult)
            nc.vector.wait_ge(sem[f"gf{Bk}"], t + 1)
            if t >= 1:
                nc.vector.wait_ge(sem[f"tcs{Bk}"], t)
            nc.vector.tensor_tensor(t1_sb[:, ds(co, 128)],
                                    acts[q][:, ds(ao + 128, 128)],
                                    c_sb[:, ds(co, 128)],
                                    op=OP.mult)
            nc.vector.tensor_tensor(c_sb[:, ds(co, 128)],
                                    t1_sb[:, ds(co, 128)],
                                    t2_sb[:, ds(co, 128)],
                                    op=OP.add).then_inc(sem[f"cdv{Bk}"], 1)
            # h = o * tanh(c) (DVE or Pool)
            hE.wait_ge(sem[f"tcs{Bk}"], t + 1)
            if t >= 2:
                hE.wait_ge(sem[f"ptr{Bk}"], t - 1)
                if Bk == 0:
                    hE.wait_ge(sem[f"sh{q}"], 16 * (t // 2))
            hE.tensor_tensor(htmp[q][:, ds(co, 128)],
                             acts[q][:, ds(ao + 256, 128)],
                             tnc[q][:, ds(co, 128)],
                             op=OP.mult).then_inc(sem[f"hdv{Bk}"], 1)

        # ================= SP =================
        if t + 3 < S:
            xgc_load(t + 3)
        nc.sync.wait_ge(sem["hdv0"], t + 1)
        nc.sync.wait_ge(sem["hdv1"], t + 1)
        nc.sync.dma_start(hout_d.ap()[t], htmp[q]
                          ).then_inc(sem[f"sh{q}"], 16)

        # phase-1 piece: DVE evac + SP out-DMA + next xT tile prefetch
        emit_piece_rest(pi)

    # ---- epilogue: drain ----
    for par in range(2):
        nc.sync.wait_ge(sem[f"sh{par}"], 16 * ((S + 1 - par) // 2))
    for par in range(3):
        nc.sync.wait_ge(sem[f"sxg{par}"], 16 * ((S - par + 2) // 3))
    nc.sync.wait_ge(sem["p1s"], 16 * NP_TOT)

    nc.compile()
    return nc


_CACHE = {}


def _get(S, use_pool=False):
    key = (S, use_pool)
    if key not in _CACHE:
        _CACHE[key] = build(S, use_pool)
    return _CACHE[key]


def _row_idx():
    """W-row index for xg/psum column (B, J, n): island (B, J) = dim chunk
    k' = 4B+J, cols [i|f|o|g~] x 128 of dims [128k', 128k'+128)."""
    idx = np.empty(4096, np.int64)
    for Bk in range(2):
        for J in range(4):
            kp = 4 * Bk + J
            for X in range(4):
                base = 2048 * Bk + 512 * J + 128 * X
                idx[base:base + 128] = GB[X] + 128 * kp + np.arange(128)
    return idx


def _prep_w(W):
    """W [4096, 1024] f32 -> [128, KH*4096] bf16, layout [p, (k, B, J, n)]."""
    w4 = W[_row_idx()].reshape(2, 4, 512, KH, 128)      # [B, J, n, k, p]
    arr = np.ascontiguousarray(w4.transpose(4, 3, 0, 1, 2)
                               ).reshape(128, KH * 4096)
    return arr.astype(BF16_NP)


def _prep_x(x, S):
    """x [16, S, 1024] f32 -> xt [16, SG, KH, 128, TSZ] bf16."""
    TSZ = min(S, 128)
    SG = S // TSZ
    xb = x.astype(BF16_NP)                                # [b, s, i]
    xb = xb.reshape(BL, SG, TSZ, KH, 128)                 # [b, g, s, k, p]
    return np.ascontiguousarray(xb.transpose(0, 1, 3, 4, 2))


def _selmats():
    sel = np.zeros((128, 64), np.float32)
    for J in range(4):
        for b in range(BL):
            sel[32 * J + b, 16 * J + b] = 1.0
    sel2 = np.eye(64, dtype=np.float32)
    return sel.astype(BF16_NP), sel2.astype(BF16_NP)


def _host_shard(inputs, S):
    fx = np.asarray(inputs["forward_x"], np.float32)[:, :S]
    bx = np.asarray(inputs["backward_x"], np.float32)[:, :S][:, ::-1]
    ridx = _row_idx()
    sel, sel2 = _selmats()
    wmaps = {}
    for d, sfx in (("f", "_f"), ("b", "_b")):
        bias = (np.asarray(inputs[f"b_ih{sfx}"], np.float32)
                + np.asarray(inputs[f"b_hh{sfx}"], np.float32))[ridx]
        wmaps[d] = {
            "wihT": _prep_w(np.asarray(inputs[f"W_ih{sfx}"], np.float32)),
            "whhT": _prep_w(np.asarray(inputs[f"W_hh{sfx}"], np.float32)),
            "biasb": np.ascontiguousarray(
                np.broadcast_to(bias.astype(BF16_NP), (128, 4096))),
        }
    maps = []
    for r in range(NCORES):
        d = "f" if r < 4 else "b"
        x = fx if d == "f" else bx
        qb = r % 4
        m = dict(wmaps[d])
        m["sel"] = sel
        m["sel2"] = sel2
        m["xt"] = _prep_x(x[16 * qb:16 * qb + 16], S)
        maps.append(m)
    return maps


def _detile(h, S):
    """[S, 128, 256] tiled -> [16, S, 1024]: h[s, 32J+b, 128B+u] = out[b, s,
    128*(4B+J)+u]."""
    a = np.asarray(h, np.float32).reshape(S, 4, 32, 2, 128)[:, :, :BL]
    return np.ascontiguousarray(a.transpose(2, 0, 3, 1, 4)).reshape(BL, S, 1024)


def run(inputs, S=S_FULL, trace=False, use_pool=False, **_):
    maps = _host_shard(inputs, S)
    nc = _get(S, use_pool)
    t0 = time.time()
    res = run_bass_kernel_spmd(nc, maps, core_ids=list(range(NCORES)),
                               trace=trace)
    wall = time.time() - t0
    outs = res.results
    fwd = np.concatenate([_detile(outs[r]["h"], S) for r in range(4)], axis=0)
    bwd = np.concatenate([_detile(outs[r]["h"], S)
                          for r in range(4, 8)], axis=0)[:, ::-1]
    return (fwd, bwd), res, wall


def kernel(**inputs):
    (fwd, bwd), _, _ = run(inputs)
    return fwd, bwd
